# revision 22
# baseline (speedup 1.0000x reference)
"""Trainium2 Bass kernel for the light-field disparity cost-volume build.

Input  x:   (2, 16, 25, 128, 128) f32  (b, c, n=angRes^2, h, w)
Output:     (2, 16, 25, 9, 128, 128) f32  (b, c, n, D, h, w)

out[b,c,(a1,a2),d,y,x] = x[b,c,(a1,a2), y + d*(2-a1), x + d*(2-a2)]
(zero outside the image), d in [-4, 4].

Pure data movement. Sharding: the 32 (b*c) slices split 4-per-core over
8 NeuronCores (data parallel, no cross-core communication).

Performance model (measured on TRN2): DMA throughput here is bound by
per-descriptor processing (~12ns/desc/path), so 512B descriptors cap a
path at ~44 GB/s while multi-KB descriptors reach ~240 GB/s. A
descriptor is a contiguous src run paired with a contiguous dst run, so:

  - tiles with column shift c==0 (d==0 or a2==2) are copied DRAM->DRAM
    straight from x with up-to-64KB descriptors (their rows are
    consecutive in x), nearly free on the descriptor budget;
  - zero-fill rows use large descriptors from an SBUF zeros region;
  - the remaining (c!=0) tiles fundamentally need one 512B descriptor
    per output row (rows live one-per-partition in SBUF, and c!=0
    breaks DRAM-side row continuity). That work goes 3/8 + 3/8 to the
    two HWDGE rings and 1/4 to gpsimd/SWDGE (measured optimum: more
    SWDGE share degrades the rings' descriptor rate via SDMA-engine
    contention faster than it helps).

SBUF staging: image row -> partition; per (slice, view) a 144-element
padded row chunk ([8 zeros][128 row][8 zeros]) so any column shift
reads its left/right zero padding as part of the row descriptor.
"""

from contextlib import ExitStack

import numpy as np

import concourse.bass as bass
import concourse.mybir as mybir
from concourse.bass import AP
from concourse.bass_utils import run_bass_kernel_spmd

F32 = mybir.dt.float32

B, C, NV, H, W = 2, 16, 25, 128, 128
A = 5          # angular resolution
MIND, MAXD = -4, 4
D = MAXD - MIND + 1
NCORES = 8
NS = (B * C) // NCORES      # slices per core = 4

CHUNK = 144                 # padded row elems per (slice, view)
ZOFF = NS * NV * CHUNK      # zeros region offset in the free dim
ZLEN = 1024                 # zeros region elems per partition
PITCH = ZOFF + ZLEN         # SBUF free elems per partition

X_V = H * W                 # input view stride (elems)
X_S = NV * X_V              # input slice stride
O_T = H * W                 # output tile stride
O_V = D * O_T               # output view stride
O_S = NV * O_V              # output slice stride

# round-robin pattern for the c!=0 interior stores over the descriptor
# generation paths (0=gpsimd, 1=sync, 2=scalar). Measured sweep: pool
# share 0 -> 577us, 1/4 -> 544us (best), 3/8 -> 547us, 1/2+ -> worse;
# SWDGE store traffic degrades the HWDGE rings' descriptor rate (13.3
# -> 16.6 ns/desc) via SDMA-engine contention, so its share must stay
# small. All large-descriptor work (loads, d=0 and a2=2 DRAM->DRAM
# copies, zero rows) lives on gpsimd to keep the rings pure.
INT_PATTERN = (1, 2, 0, 1, 2, 1, 2, 0)


def _build_nc():
    nc = bass.Bass()
    x = nc.dram_tensor("x", [NS, NV, H, W], F32, kind="ExternalInput")
    out = nc.dram_tensor("out", [NS, NV, D, H, W], F32, kind="ExternalOutput")

    # (s, a1, d, a2pair) jobs for the c!=0 interiors: d != 0, a2 in {0,1},{3,4}
    jobs = []
    for s in range(NS):
        for a1 in range(A):
            for d in range(MIND, MAXD + 1):
                if d == 0:
                    continue
                for a2_0 in (0, 3):
                    jobs.append((s, a1, d, a2_0))
    shares = ([], [], [])
    for i, job in enumerate(jobs):
        shares[INT_PATTERN[i % len(INT_PATTERN)]].append(job)
    # the scalar (ACT) ring runs ~134ns/DMA slower than sync (its
    # DGE_DMA_DELAY); shift two jobs over so both rings finish together
    shares[1].extend(shares[2][-2:])
    del shares[2][-2:]

    zero_jobs = [
        (a1, d)
        for a1 in range(A)
        for d in range(MIND, MAXD + 1)
        if d * (A // 2 - a1) != 0
    ]

    with (
        ExitStack() as stack,
        nc.sbuf_tensor([128, PITCH], F32) as buf,
        nc.semaphore("msem") as msem,
        nc.semaphore("s1") as s1,
        nc.semaphore("s2") as s2,
        nc.semaphore("s3") as s3,
        nc.semaphore("zsem") as zsem,
        nc.semaphore("dsem") as dsem,
        nc.Block() as block,
    ):
        lsems = [stack.enter_context(nc.semaphore(f"lsem{s}")) for s in range(NS)]

        @block.vector
        def _(vector):
            # zero the column pads + the zeros region. Chunk k's tail pad
            # and chunk k+1's head pad form one contiguous 16-elem run.
            vector.memset(AP(buf, 0, [[PITCH, 128], [1, 8]]), 0.0).then_inc(msem, 1)
            vector.memset(
                AP(buf, 136, [[PITCH, 128], [CHUNK, NS * NV - 1], [1, 16]]), 0.0
            ).then_inc(msem, 1)
            vector.memset(
                AP(buf, ZOFF - 8, [[PITCH, 128], [1, 8 + ZLEN]]), 0.0
            ).then_inc(msem, 1)

        def interior_store(engine, job, sem):
            s, a1, d, a2_0 = job
            r = d * (A // 2 - a1)
            nr = H - abs(r)
            di = d - MIND
            v0 = NV * s + A * a1
            src_off = (max(0, r) * PITCH + CHUNK * v0 + 8 + 2 * d
                       + a2_0 * (CHUNK - d))
            dst_off = (s * O_S + (A * a1 * D + di) * O_T + max(0, -r) * W
                       + a2_0 * O_V)
            engine.dma_start(
                out=AP(out, dst_off, [[W, nr], [O_V, 2], [1, W]]),
                in_=AP(buf, src_off, [[PITCH, nr], [CHUNK - d, 2], [1, W]]),
            ).then_inc(sem, 16)

        def store_stream(engine, share, sem):
            engine.wait_ge(msem, 3)
            loaded = 0
            for job in share:
                s = job[0]
                if s >= loaded:
                    loaded = s + 1
                    engine.wait_ge(lsems[s], 16)
                interior_store(engine, job, sem)
            engine.wait_ge(sem, 16 * len(share))

        @block.gpsimd
        def _(gpsimd):
            # loads: one DMA per slice, x[s] -> per-(s,v) padded chunks
            for s in range(NS):
                gpsimd.dma_start(
                    out=AP(buf, CHUNK * NV * s + 8, [[PITCH, 128], [CHUNK, NV], [1, W]]),
                    in_=AP(x, s * X_S, [[W, H], [X_V, NV], [1, W]]),
                ).then_inc(lsems[s], 16)
            # big-descriptor extras next, then gpsimd's interior share.
            # (Ordering measured: extras-first 544us vs extras-last 551us;
            # the ring-contention integral of pool traffic is roughly
            # order-independent, extras-first starts their HBM reads
            # while the rings are still ramping on slice-0 interiors.)
            # d=0 tiles: straight copy of every view, DRAM->DRAM
            gpsimd.dma_start(
                out=AP(out, (0 - MIND) * O_T, [[O_V, NS * NV], [1, X_V]]),
                in_=AP(x, 0, [[X_V, NS * NV], [1, X_V]]),
                max_dma_last_dim=8192,
            ).then_inc(dsem, 16)
            # a2==2, d!=0 tiles: row-shifted copy, rows contiguous in x
            n_d = 1
            for a1 in range(A):
                for d in range(MIND, MAXD + 1):
                    if d == 0:
                        continue
                    r = d * (A // 2 - a1)
                    nr = H - abs(r)
                    v = A * a1 + 2
                    di = d - MIND
                    src_off = v * X_V + max(0, r) * W
                    dst_off = (v * D + di) * O_T + max(0, -r) * W
                    gpsimd.dma_start(
                        out=AP(out, dst_off, [[O_S, NS], [1, nr * W]]),
                        in_=AP(x, src_off, [[X_S, NS], [1, nr * W]]),
                    ).then_inc(dsem, 16)
                    n_d += 1
            # zero-row stores, large descriptors from the zeros region
            gpsimd.wait_ge(msem, 3)
            for a1, d in zero_jobs:
                r = d * (A // 2 - a1)
                nz = abs(r)
                di = d - MIND
                dst_off = (A * a1 * D + di) * O_T + ((H - r) * W if r > 0 else 0)
                gpsimd.dma_start(
                    out=AP(out, dst_off, [[O_S, NS], [O_V, A], [1, W * nz]]),
                    in_=AP(buf, ZOFF, [[PITCH, NS * A], [1, W * nz]]),
                ).then_inc(zsem, 16)
            store_stream(gpsimd, shares[0], s1)
            gpsimd.wait_ge(zsem, 16 * len(zero_jobs))
            gpsimd.wait_ge(dsem, 16 * n_d)

        @block.sync
        def _(sync):
            store_stream(sync, shares[1], s2)

        @block.scalar
        def _(scalar):
            store_stream(scalar, shares[2], s3)

    return nc


_NC = None


def _get_nc():
    global _NC
    if _NC is None:
        _NC = _build_nc()
    return _NC


def kernel(x: np.ndarray) -> np.ndarray:
    assert x.shape == (B, C, NV, H, W), x.shape
    xs = np.ascontiguousarray(x.astype(np.float32, copy=False)).reshape(
        B * C, NV, H, W
    )
    in_maps = [{"x": xs[NS * k : NS * (k + 1)]} for k in range(NCORES)]
    res = run_bass_kernel_spmd(_get_nc(), in_maps, core_ids=list(range(NCORES)))
    out = np.concatenate([r["out"] for r in res.results], axis=0)
    return out.reshape(B, C, NV, D, H, W)
